# revision 6
# baseline (speedup 1.0000x reference)
"""Trainium2 Bass kernel for the 3-layer LCN/GNN network, PE-dense strategy
with multi-way split AllGather/compute overlap and L0 row compaction.

As kernel_g, generalized: layer 0 computes its output columns in EIGHT
eighth-passes (layer 1 in two half-passes), so the first quarter's
bias/ReLU/transpose/AllGather overlaps the remaining three quarters'
PE streaming, and each gathered piece's restage + next-layer k-tiles
start earlier. Finer column splits also deepen the L0 contraction-row
compaction: an eighth's S_0 columns hit only ~12% of the 16384 rows
(mean hits/row 0.125), so each (core, eighth) streams just KT0Q=20
k-tiles of compacted rows. Per-piece transposed outputs are staged in
one tile and written to the DRAM shard with a single DMA.
"""

import os
import sys
import types

import numpy as np

try:  # pragma: no cover
    import antenv.axon_hooks  # noqa: F401
except Exception:
    _m = types.ModuleType("antenv.axon_hooks")
    _m.get_axon_ntff_profile_hook = lambda: None
    sys.modules["antenv.axon_hooks"] = _m

B, IN_DIM, K = 256, 16384, 16
DIMS = [8192, 4096, 2048]
PREV = [IN_DIM] + DIMS[:-1]
OUT_DIM = 3
N_CORES = 8
P = 128
HL = [8, 2, 1]     # column-split count per layer
KT0Q = 20          # compacted L0 k-tiles per eighth (mean 15.1, +15 sigma)
JJ = 4             # k-tiles per streamed S slab (256KB)

_cache = {}


def _perm(dim, H):
    """Row order of the gathered table for a layer with `dim` total nodes
    split H ways: piece-major [cores x piece0 | cores x piece1 | ...]."""
    sh = dim // N_CORES
    blk = sh // H
    return np.asarray(
        [c * sh + q * blk + j
         for q in range(H) for c in range(N_CORES) for j in range(blk)],
        dtype=np.int64)


def _build(reps: int = 1):
    import concourse.tile as tile
    from concourse import bacc, mybir

    nc = bacc.Bacc("TRN2", target_bir_lowering=False, debug=False,
                   num_devices=N_CORES)
    f32 = mybir.dt.float32
    bf16 = mybir.dt.bfloat16

    shard = [d // N_CORES for d in DIMS]  # 1024, 512, 256
    groups = [list(range(N_CORES))]
    kts = [KT0Q, PREV[1] // P, PREV[2] // P]  # k-tiles per pass

    t0 = nc.dram_tensor("t0", [HL[0] * KT0Q * P, B], bf16,
                        kind="ExternalInput")
    tn, shd = [], []
    for l in range(2):
        H = HL[l]
        tn.append([nc.dram_tensor(f"t{l + 1}p{h}", [DIMS[l] // H, B], bf16,
                                  addr_space="Shared") for h in range(H)])
        shd.append([nc.dram_tensor(f"sh{l + 1}p{h}", [shard[l] // H, B], bf16)
                    for h in range(H)])

    s_d = [nc.dram_tensor(f"s{l}", [HL[l] * kts[l] * P, shard[l] // HL[l]],
                          bf16, kind="ExternalInput") for l in range(3)]
    bias_d = [nc.dram_tensor(f"bias{l}", [1, shard[l]], bf16,
                             kind="ExternalInput") for l in range(3)]
    ones_d = nc.dram_tensor("ones", [1, P], bf16, kind="ExternalInput")
    fcw_d = nc.dram_tensor("fcw", [P, 2 * OUT_DIM], bf16, kind="ExternalInput")
    out_d = nc.dram_tensor("out", [OUT_DIM, B], f32, kind="ExternalOutput")

    with tile.TileContext(nc) as tc:
        with (
            tc.tile_pool(name="const", bufs=1) as cpool,
            tc.tile_pool(name="wstr", bufs=6) as wpool,
            tc.tile_pool(name="acts", bufs=2) as apool,
            tc.tile_pool(name="psum", bufs=1, space="PSUM") as ppool,
        ):
            a0 = cpool.tile([P, HL[0] * KT0Q, B], bf16, tag="a0")
            a1 = cpool.tile([P, DIMS[0] // P, B], bf16, tag="a1")
            a2 = cpool.tile([P, DIMS[1] // P, B], bf16, tag="a2")
            acts = [a0, a1, a2]
            nc.sync.dma_start(
                out=a0[:], in_=t0.ap().rearrange("(c p) b -> p c b", p=P))
            ones = cpool.tile([1, P], bf16, tag="ones")
            nc.sync.dma_start(out=ones[:], in_=ones_d.ap())
            bias_sb = []
            for l in range(3):
                bt = cpool.tile([1, shard[l]], bf16, tag=f"bias{l}")
                nc.sync.dma_start(out=bt[:], in_=bias_d[l].ap())
                bias_sb.append(bt)
            fcw_sb = cpool.tile([P, 2 * OUT_DIM], bf16, tag="fcw")
            nc.sync.dma_start(out=fcw_sb[:], in_=fcw_d.ap())
            act2T = cpool.tile([P, 2, B], bf16, tag="act2T")

            def emit_net():
                for l in range(3):
                    kt = kts[l]
                    H = HL[l]
                    nsh = shard[l] // H
                    at = acts[l]
                    for h in range(H):
                        pss = {}
                        for m in range(2):
                            pss[m] = ppool.tile([P, nsh], f32, tag=f"ps{m}",
                                                name=f"ps{m}")
                        for kj in range(kt // JJ):
                            st = wpool.tile([P, JJ, nsh], bf16, tag="S")
                            r0 = (h * kt + kj * JJ) * P
                            nc.sync.dma_start(
                                out=st[:],
                                in_=s_d[l][r0:r0 + JJ * P, :].rearrange(
                                    "(j p) n -> p j n", p=P))
                            for jj in range(JJ):
                                ki = kj * JJ + jj
                                kia = h * kt + ki if l == 0 else ki
                                for m in range(2):
                                    nc.tensor.matmul(
                                        out=pss[m][:],
                                        lhsT=at[:, kia, m * 128:(m + 1) * 128],
                                        rhs=st[:, jj, :],
                                        start=(ki == 0),
                                        stop=False,
                                    )
                        arows = []
                        for m in range(2):
                            nc.tensor.matmul(
                                out=pss[m][:],
                                lhsT=ones[:],
                                rhs=bias_sb[l][:, h * nsh:(h + 1) * nsh],
                                start=False,
                                stop=True,
                            )
                            arow = apool.tile([P, nsh], bf16, tag=f"ar{m}")
                            nc.scalar.activation(
                                out=arow[:],
                                in_=pss[m][:],
                                func=mybir.ActivationFunctionType.Relu,
                                scale=1.0,
                            )
                            arows.append(arow)
                        nt = nsh // P
                        if l < 2:
                            dst = apool.tile([P, nt, B], bf16, tag="T")
                        for n in range(nt):
                            if l < 2:
                                d0 = dst[:, n, 0:128]
                                d1 = dst[:, n, 128:256]
                            else:
                                g = h * nt + n
                                d0 = act2T[:, g, 0:128]
                                d1 = act2T[:, g, 128:256]
                            nc.sync.dma_start(
                                out=d0, in_=arows[0][:, n * P:(n + 1) * P],
                                transpose=True)
                            nc.sync.dma_start(
                                out=d1, in_=arows[1][:, n * P:(n + 1) * P],
                                transpose=True)
                        if l < 2:
                            nc.sync.dma_start(
                                out=shd[l][h].ap().rearrange(
                                    "(n p) b -> p n b", p=P),
                                in_=dst[:])
                            nc.gpsimd.collective_compute(
                                "AllGather",
                                mybir.AluOpType.bypass,
                                groups,
                                ins=[shd[l][h].ap()],
                                outs=[tn[l][h].ap()],
                            )
                            ktn_h = DIMS[l] // H // P
                            nc.sync.dma_start(
                                out=acts[l + 1][:, h * ktn_h:(h + 1) * ktn_h, :],
                                in_=tn[l][h].ap().rearrange(
                                    "(c p) b -> p c b", p=P))

                ps = ppool.tile([OUT_DIM, B], f32, tag="fc")
                for t in range(2):
                    nc.tensor.matmul(
                        out=ps[:],
                        lhsT=fcw_sb[:, t * OUT_DIM:(t + 1) * OUT_DIM],
                        rhs=act2T[:, t, :],
                        start=(t == 0),
                        stop=(t == 1),
                    )
                fin = apool.tile([OUT_DIM, B], f32, tag="fin")
                nc.vector.tensor_copy(out=fin[:], in_=ps[:])
                nc.sync.dma_start(out=out_d.ap(), in_=fin[:])

            for r in range(reps):
                if r:
                    tc.strict_bb_all_engine_barrier()
                emit_net()

    nc.compile()
    return nc


def _prep_inputs(inputs):
    import ml_dtypes

    shard = [d // N_CORES for d in DIMS]
    x = np.asarray(inputs["x"], dtype=np.float32)
    t0 = np.ascontiguousarray(x.T).astype(ml_dtypes.bfloat16)
    fcw = np.asarray(inputs["fc_w"], dtype=np.float32)
    ones = np.ones((1, P), dtype=ml_dtypes.bfloat16)
    perms = [None, _perm(DIMS[0], HL[0]), _perm(DIMS[1], HL[1])]

    in_maps = []
    for m in range(N_CORES):
        im = {"ones": ones}
        for l, d in enumerate(DIMS):
            knn = np.asarray(inputs[f"knn{l}"], dtype=np.int64)
            w = np.asarray(inputs[f"w{l}"], dtype=np.float32)
            b = np.asarray(inputs[f"b{l}"], dtype=np.float32).reshape(d)
            lo = m * shard[l]
            nodes = np.arange(lo, lo + shard[l])
            s32 = np.zeros((PREV[l], shard[l]), dtype=np.float32)
            rows = knn[nodes].ravel()
            cols = np.repeat(np.arange(shard[l]), K)
            np.add.at(s32, (rows, cols), w[nodes].ravel())
            if perms[l] is not None:
                s32 = s32[perms[l]]  # match the gathered-table row order
            H = HL[l]
            nsh = shard[l] // H
            if l == 0:
                sblk, tblk = [], []
                for h in range(H):
                    sub = s32[:, h * nsh:(h + 1) * nsh]
                    used = np.flatnonzero(sub.any(axis=1))
                    assert len(used) <= KT0Q * P, len(used)
                    sc = np.zeros((KT0Q * P, nsh), dtype=np.float32)
                    sc[:len(used)] = sub[used]
                    tc_ = np.zeros((KT0Q * P, B), dtype=ml_dtypes.bfloat16)
                    tc_[:len(used)] = t0[used]
                    sblk.append(sc)
                    tblk.append(tc_)
                im["t0"] = np.ascontiguousarray(np.concatenate(tblk, axis=0))
                s32 = np.concatenate(sblk, axis=0)
            elif H > 1:
                s32 = np.concatenate(
                    [s32[:, h * nsh:(h + 1) * nsh] for h in range(H)], axis=0)
            im[f"s{l}"] = np.ascontiguousarray(s32).astype(ml_dtypes.bfloat16)
            im[f"bias{l}"] = b[lo:lo + shard[l]].reshape(1, -1).astype(
                ml_dtypes.bfloat16)
        cols = fcw[:, m * 256:(m + 1) * 256].T
        im["fcw"] = np.ascontiguousarray(
            cols.reshape(2, P, OUT_DIM).transpose(1, 0, 2).reshape(P, 2 * OUT_DIM)
        ).astype(ml_dtypes.bfloat16)
        in_maps.append(im)
    return in_maps


def kernel(**inputs) -> np.ndarray:
    from concourse.bass_utils import run_bass_kernel_spmd

    reps = int(os.environ.get("KERNEL_REPS", "1"))
    key = ("nc", reps)
    if key not in _cache:
        _cache[key] = _build(reps)
    nc = _cache[key]

    in_maps = _prep_inputs(inputs)
    res = run_bass_kernel_spmd(nc, in_maps, list(range(N_CORES)))
    if res.exec_time_ns is not None:
        print(f"HW exec time: {res.exec_time_ns} ns")
    acc = np.zeros((OUT_DIM, B), dtype=np.float32)
    for r in res.results:
        acc += r["out"]
    fc_b = np.asarray(inputs["fc_b"], dtype=np.float32)
    return (acc.T + fc_b[None, :]).astype(np.float32)


if __name__ == "__main__":
    sys.path.insert(0, "/root/problem")
    inputs = dict(np.load("/root/problem/inputs.npz"))
    expected = np.load("/root/problem/expected.npy")
    actual = kernel(**inputs)
    err = np.abs(actual - expected)
    scale = np.abs(expected).max()
    print(f"absmax err: {err.max():.6g}  scale: {scale:.6g}")
    print(f"Relative error: {err.max() / scale:.6g}")
